# revision 36
# baseline (speedup 1.0000x reference)
"""Multi-head attention (B=2, N=2048, C=1024, H=16, D=64) on 8 TRN2 NeuronCores.

Sharding: core c = (batch b = c//4) x (head-group g = c%4 -> heads 4g..4g+3).
Data parallel on B, tensor parallel on heads; fp16 ReduceScatter of the
out-projection partials within each 4-core batch group.

Everything on device stays transposed ([channel, position]); the host
pre-transposes inputs and post-transposes the output.
"""

import numpy as np
import ml_dtypes

import concourse.bacc as bacc
import concourse.tile as tile
import concourse.mybir as mybir
from concourse.bass_utils import run_bass_kernel_spmd

B, N, C, H = 2, 2048, 1024, 16
D = C // H          # 64
HL = H // 4         # 4 heads per core
CL = HL * D         # 256 local channels
N_CORES = 8
GROUPS = [[0, 1, 2, 3], [4, 5, 6, 7]]

F32 = mybir.dt.float32
BF16 = mybir.dt.float16
BF = np.float16

KC = C // 128       # 8  K-chunks of the input channel dim
NI = N // 512       # 4  512-wide i-chunks
NJ = N // 128       # 16 128-row j-chunks
IH = N // 1024      # 2  i-halves (attention granularity / RS chunks)


def build_kernel(n_cores=N_CORES, groups=GROUPS):
    group_size = len(groups[0])
    rs_out_rows = C // group_size

    nc = bacc.Bacc("TRN2", target_bir_lowering=False, debug=False,
                   num_devices=n_cores)

    xT = nc.declare_dram_parameter("xT", [C, N], BF16, isOutput=False)
    cos2 = nc.declare_dram_parameter("cos2", [128, N], BF16, isOutput=False)
    sin2s = nc.declare_dram_parameter("sin2s", [128, N], BF16, isOutput=False)
    wqkT = nc.declare_dram_parameter("wqkT", [C, 2 * CL], BF16, isOutput=False)
    bqk = nc.declare_dram_parameter("bqk", [2 * CL, 1], F32, isOutput=False)
    wvT = nc.declare_dram_parameter("wvT", [C, CL], BF16, isOutput=False)
    wprojT = nc.declare_dram_parameter("wprojT", [CL, C], BF16, isOutput=False)
    beff = nc.declare_dram_parameter("beff", [rs_out_rows, 1], F32, isOutput=False)
    out = nc.declare_dram_parameter("out", [rs_out_rows, N], F32, isOutput=True)

    with tile.TileContext(nc) as tc:
        with tc.tile_pool(name="dram", bufs=1, space="DRAM") as dram, \
             tc.tile_pool(name="sbuf", bufs=1) as sb, \
             tc.tile_pool(name="psum", bufs=1, space="PSUM") as ps:

            # ---- load inputs (wqk/xb interleaved so the qk matmuls can start
            # before the full x transfer lands) ----
            xb, wqk_sb = [], []
            for kc in range(KC):
                t = sb.tile([128, 2 * CL], BF16, name=f"wqk{kc}", tag=f"wqk{kc}")
                nc.scalar.dma_start(t[:], wqkT.ap()[128 * kc:128 * (kc + 1), :])
                wqk_sb.append(t)
                t = sb.tile([128, N], BF16, name=f"xb{kc}", tag=f"xb{kc}")
                eng = nc.sync if kc % 2 == 0 else nc.gpsimd
                eng.dma_start(t[:], xT.ap()[128 * kc:128 * (kc + 1), :])
                xb.append(t)
            wv_sb = []
            for kc in range(KC):
                t = sb.tile([128, CL], BF16, name=f"wv{kc}", tag=f"wv{kc}")
                nc.sync.dma_start(t[:], wvT.ap()[128 * kc:128 * (kc + 1), :])
                wv_sb.append(t)
            cos_sb = sb.tile([128, N], BF16, name="cos_sb", tag="cos_sb")
            nc.gpsimd.dma_start(cos_sb[:], cos2.ap())
            sin_sb = sb.tile([128, N], BF16, name="sin_sb", tag="sin_sb")
            nc.gpsimd.dma_start(sin_sb[:], sin2s.ap())
            bqk_sb = []
            for m in range(4):
                t = sb.tile([128, 1], F32, name=f"bqk{m}", tag=f"bqk{m}")
                nc.sync.dma_start(t[:], bqk.ap()[128 * m:128 * (m + 1), :])
                bqk_sb.append(t)
            wproj_sb = []
            for p in range(2):
                t = sb.tile([128, C], BF16, name=f"wproj{p}", tag=f"wproj{p}")
                nc.sync.dma_start(t[:], wprojT.ap()[128 * p:128 * (p + 1), :])
                wproj_sb.append(t)
            beff_sb = []
            for m in range(rs_out_rows // 128):
                t = sb.tile([128, 1], F32, name=f"beff{m}", tag=f"beff{m}")
                nc.sync.dma_start(t[:], beff.ap()[128 * m:128 * (m + 1), :])
                beff_sb.append(t)

            # ---- qk projection + RoPE ----
            # chunk m rows: m=0:[q_h0,q_h1] m=1:[q_h2,q_h3] m=2:[k_h0,k_h1] m=3:[k_h2,k_h3]
            # so q and k of head h sit at the same partition offset 64*(h%2).
            # k of each head lands in its own zero-padded [128, N] tile so the
            # scores matmul can contract over K=128 (16-bit matmuls run at
            # half rate for K=64 -- zero rows buy back the full rate).
            q_r = []      # 2 tiles: [q_h0,q_h1], [q_h2,q_h3]
            k_t = []      # 4 tiles: k_h at rows 64*(h%2), zeros elsewhere
            for h in range(4):
                kt = sb.tile([128, N], BF16, name=f"ktile{h}", tag=f"ktile{h}")
                z = slice(0, 64) if h % 2 == 1 else slice(64, 128)
                nc.vector.memset(kt[z, :], 0.0)
                k_t.append(kt)
            swap_mask = [i ^ 1 for i in range(32)]
            # kc-outer accumulation so the first matmul only needs the first
            # x/w chunk off DMA; 2 PSUM tiles hold the 4 m-accumulators
            qks_t = [sb.tile([128, N], BF16, name=f"qks{m}", tag=f"qks{m}")
                     for m in range(4)]
            for n in range(NI):
                accs = [ps.tile([128, 1024], F32, name=f"qacc{n}_{a}",
                                tag="sc", bufs=2) for a in range(2)]
                for kc in range(KC):
                    for m in range(4):
                        nc.tensor.matmul(
                            accs[m // 2][:, 512 * (m % 2):512 * (m % 2 + 1)],
                            wqk_sb[kc][:, 128 * m:128 * (m + 1)],
                            xb[kc][:, 512 * n:512 * (n + 1)],
                            start=(kc == 0), stop=(kc == KC - 1))
                for m in range(4):
                    nc.scalar.activation(
                        qks_t[m][:, 512 * n:512 * (n + 1)],
                        accs[m // 2][:, 512 * (m % 2):512 * (m % 2 + 1)],
                        mybir.ActivationFunctionType.Identity,
                        bias=bqk_sb[m][:])
            for m in range(4):
                qks = qks_t[m]
                # RoPE: qk' = qks*cos2 + shift(qks)*sin2s
                # (pair-swap of adjacent partitions via DVE stream shuffle)
                shf = sb.tile([128, N], BF16, name=f"shf{m}", tag="shf", bufs=2)
                nc.vector.stream_shuffle(shf[:], qks[:], swap_mask)
                t2 = sb.tile([128, N], BF16, name=f"ropetmp{m}", tag="ropetmp", bufs=2)
                nc.vector.tensor_mul(t2[:], shf[:], sin_sb[:])
                if m < 2:
                    qkr = sb.tile([128, N], BF16, name=f"qkr{m}", tag=f"qkr{m}")
                    nc.vector.tensor_mul(qkr[:], qks[:], cos_sb[:])
                    nc.vector.tensor_add(qkr[:], qkr[:], t2[:])
                    q_r.append(qkr)
                else:
                    t1 = sb.tile([128, N], BF16, name=f"ropetc{m}", tag="ropetc",
                                 bufs=2)
                    nc.vector.tensor_mul(t1[:], qks[:], cos_sb[:])
                    h0, h1 = 2 * (m - 2), 2 * (m - 2) + 1
                    nc.vector.tensor_add(k_t[h0][0:64, :], t1[0:64, :],
                                         t2[0:64, :])
                    nc.vector.tensor_add(k_t[h1][64:128, :], t1[64:128, :],
                                         t2[64:128, :])

            # ---- v projection (natural [j, ch] layout, ones col appended per head) ----
            vaug = []
            for jc in range(NJ):
                pv = ps.tile([128, CL], F32, name=f"pv{jc}", tag="sc", bufs=2)
                for kc in range(KC):
                    nc.tensor.matmul(
                        pv[:],
                        xb[kc][:, 128 * jc:128 * (jc + 1)],
                        wv_sb[kc][:],
                        start=(kc == 0), stop=(kc == KC - 1))
                va = sb.tile([128, HL * (D + 1)], BF16, name=f"vaug{jc}",
                             tag=f"vaug{jc}")
                nc.vector.memset(va[:, D::D + 1], 1.0)
                nc.scalar.activation(
                    va.rearrange("p (h e) -> p h e", e=D + 1)[:, :, 0:D],
                    pv.rearrange("p (h e) -> p h e", e=D)[:, :, :],
                    mybir.ActivationFunctionType.Copy)
                vaug.append(va)

            # per-partition bias AP used to shift scores before fp16 exp
            eshift = sb.tile([128, 1], F32, name="eshift", tag="eshift")
            nc.vector.memset(eshift[:], -16.0)
            # K=1 ones row used to broadcast denominators across partitions
            ones64 = sb.tile([1, 64], BF16, name="ones64", tag="ones64")
            nc.vector.memset(ones64[:], 1.0)

            # ---- attention + projection + RS, per i-half ----
            rs_outs = []
            for ih in range(IH):
                i0 = 1024 * ih
                o_pair = [sb.tile([128, 1024], BF16, name=f"opair{ih}_{p}",
                                  tag=f"opair{p}", bufs=2) for p in range(2)]
                for hl in range(4):
                    qT = q_r[hl // 2]
                    kT = k_t[hl]
                    oacc = ps.tile([65, 1024], F32, name=f"oacc{ih}_{hl}",
                                   tag="oacc", bufs=2)
                    exs = []

                    def emit_o(jc):
                        for q in range(2):
                            nc.tensor.matmul(
                                oacc[:, 512 * q:512 * (q + 1)],
                                vaug[jc][:, (D + 1) * hl:(D + 1) * (hl + 1)],
                                exs[jc][:, 512 * q:512 * (q + 1)],
                                start=(jc == 0), stop=(jc == NJ - 1))

                    for jc in range(NJ):
                        sc = ps.tile([128, 1024], F32, name=f"sc{ih}_{hl}_{jc}",
                                     tag="sc", bufs=2)
                        for q in range(2):
                            nc.tensor.matmul(
                                sc[:, 512 * q:512 * (q + 1)],
                                kT[:, 128 * jc:128 * (jc + 1)],
                                qT[:, i0 + 512 * q:i0 + 512 * (q + 1)],
                                start=True, stop=True)
                        ex = sb.tile([128, 1024], BF16, name=f"ex{ih}_{hl}_{jc}",
                                     tag="ex", bufs=3)
                        # bias shifts all scores so fp16 exp can't overflow
                        # (softmax is shift-invariant, cancels in num/den)
                        nc.scalar.activation(ex[:], sc[:],
                                             mybir.ActivationFunctionType.Exp,
                                             scale=float(1.0 / np.sqrt(D)),
                                             bias=eshift[:])
                        exs.append(ex)
                        # software pipeline: o-matmuls lag one j-chunk so the
                        # PE never sits waiting on the exp of the current one
                        if jc >= 1:
                            emit_o(jc - 1)
                    emit_o(NJ - 1)
                    # normalize: o[:, i] / den[i]
                    # broadcast den across partitions with a K=1 matmul
                    # (no DMA: DMA triggers on any engine queue can block it
                    # while collective SDMA traffic is in flight), then
                    # reciprocal+mul on 64 partitions
                    den = sb.tile([1, 1024], BF16, name=f"den{ih}_{hl}",
                                  tag="den", bufs=2)
                    nc.scalar.activation(den[:], oacc[64:65, :],
                                         mybir.ActivationFunctionType.Copy)
                    rb = ps.tile([64, 1024], F32, name=f"rb{ih}_{hl}",
                                 tag="sc", bufs=2)
                    for q in range(2):
                        nc.tensor.matmul(rb[:, 512 * q:512 * (q + 1)],
                                         ones64[:],
                                         den[:, 512 * q:512 * (q + 1)],
                                         start=True, stop=True)
                    rr = sb.tile([64, 1024], F32, name=f"rr{ih}_{hl}", tag="rr",
                                 bufs=2)
                    nc.vector.reciprocal_approx_fast(rr[:], rb[:])
                    nc.vector.tensor_mul(
                        o_pair[hl // 2][64 * (hl % 2):64 * (hl % 2) + 64, :],
                        oacc[0:64, :], rr[:])

                # out-projection partial for this i-half
                rs_in = dram.tile([C, 1024], BF16, name=f"rsin{ih}", tag=f"rsin{ih}")
                for n2 in range(2):
                    isl = slice(512 * n2, 512 * (n2 + 1))
                    for mc in range(KC):
                        pp = ps.tile([128, 512], F32, name=f"pp{ih}_{n2}_{mc}",
                                     tag="sc" if mc % 2 == 0 else "oacc",
                                     bufs=2)
                        for p in range(2):
                            nc.tensor.matmul(
                                pp[:],
                                wproj_sb[p][:, 128 * mc:128 * (mc + 1)],
                                o_pair[p][:, isl],
                                start=(p == 0), stop=(p == 1))
                        po = sb.tile([128, 512], BF16, name=f"po{ih}_{n2}_{mc}",
                                     tag="po", bufs=4)
                        # alternate evict engine so PSUM slots recycle 2x faster
                        if mc % 2 == 0:
                            nc.vector.tensor_copy(po[:], pp[:])
                        else:
                            nc.scalar.activation(
                                po[:], pp[:],
                                mybir.ActivationFunctionType.Copy)
                        nc.sync.dma_start(
                            rs_in[128 * mc:128 * (mc + 1), isl], po[:])
                rs_out = dram.tile([rs_out_rows, 1024], BF16, name=f"rsout{ih}",
                                   tag=f"rsout{ih}")
                nc.gpsimd.collective_compute(
                    "ReduceScatter", mybir.AluOpType.add,
                    replica_groups=groups,
                    ins=[rs_in[:]], outs=[rs_out[:]])
                rs_outs.append(rs_out)

            # ---- RS result + bias -> output ----
            # gpsimd DMA queue: keeps RS-dependent reads out of the sync
            # queue so they can't head-of-line-block compute-feeding DMAs
            for ih in range(IH):
                for m in range(rs_out_rows // 128):
                    rbk = sb.tile([128, 1024], BF16, name=f"rbk{ih}_{m}",
                                  tag="rbk", bufs=2)
                    nc.gpsimd.dma_start(
                        rbk[:], rs_outs[ih][128 * m:128 * (m + 1), :])
                    fo = sb.tile([128, 1024], F32, name=f"fo{ih}_{m}", tag="fo",
                                 bufs=2)
                    nc.vector.tensor_scalar_add(fo[:], rbk[:], beff_sb[m][:])
                    nc.gpsimd.dma_start(
                        out.ap()[128 * m:128 * (m + 1),
                                 1024 * ih:1024 * (ih + 1)], fo[:])

    nc.compile()
    return nc


def shard_inputs(x, rope, w_qkv, b_qkv, w_proj, b_proj,
                 n_cores=N_CORES, group_size=4):
    """Per-core input maps. Host-side transposes/casts are part of sharding."""
    rs_out_rows = C // group_size
    # fold the v-bias through the projection into an effective output bias
    b_v = b_qkv[2 * C:3 * C]
    b_eff = (b_proj + b_v @ w_proj.T).astype(np.float32)   # [C]

    in_maps = []
    for c in range(n_cores):
        b = (c // group_size) % B
        g = c % group_size
        heads = range(HL * g, HL * g + HL)

        xTb = np.ascontiguousarray(x[b].T).astype(BF)            # [C, N]

        cosT = rope[b].T[:D, :]                                   # [64, N]
        sinT = rope[b].T[D:, :]
        cos2 = np.vstack([cosT, cosT]).astype(BF)                 # [128, N]
        sgn = np.where(np.arange(128) % 2 == 0, -1.0, 1.0)[:, None]
        sin2s = (np.vstack([sinT, sinT]) * sgn).astype(BF)        # [128, N]

        # qk weight rows ordered [q_h0..q_h3, k_h0..k_h3]
        qk_rows = []
        bqk_rows = []
        for h in heads:
            qk_rows.append(w_qkv[D * h:D * (h + 1), :])           # q rows
            bqk_rows.append(b_qkv[D * h:D * (h + 1)])
        for h in heads:
            qk_rows.append(w_qkv[C + D * h:C + D * (h + 1), :])   # k rows
            bqk_rows.append(b_qkv[C + D * h:C + D * (h + 1)])
        wqk = np.vstack(qk_rows)                                  # [512, C]
        wqkT = np.ascontiguousarray(wqk.T).astype(BF)             # [C, 512]
        bqk_v = np.concatenate(bqk_rows).astype(np.float32)[:, None]

        h0 = HL * g
        wv = w_qkv[2 * C + D * h0:2 * C + D * h0 + CL, :]          # [256, C]
        wvT = np.ascontiguousarray(wv.T).astype(BF)                # [C, 256]

        wp = w_proj[:, D * h0:D * h0 + CL]                         # [C, 256]
        wprojT = np.ascontiguousarray(wp.T).astype(BF)             # [256, C]

        r = c % group_size
        beff_shard = b_eff[rs_out_rows * r:rs_out_rows * (r + 1)].astype(
            np.float32)[:, None]

        in_maps.append({
            "xT": xTb, "cos2": cos2, "sin2s": sin2s,
            "wqkT": wqkT, "bqk": bqk_v, "wvT": wvT,
            "wprojT": wprojT, "beff": beff_shard,
        })
    return in_maps


def assemble(results, n_cores=N_CORES, group_size=4):
    rs_out_rows = C // group_size
    out = np.empty((B, N, C), dtype=np.float32)
    for c in range(n_cores):
        b = (c // group_size) % B
        r = c % group_size
        outT_shard = results[c]["out"]                 # [rs_out_rows, N] f32
        out[b, :, rs_out_rows * r:rs_out_rows * (r + 1)] = outT_shard.T
    return out


_NC_CACHE = {}


def _get_nc():
    if "nc" not in _NC_CACHE:
        _NC_CACHE["nc"] = build_kernel()
    return _NC_CACHE["nc"]


def _run(inputs, trace=False, tmpdir=None):
    nc = _get_nc()
    in_maps = shard_inputs(**inputs)
    res = run_bass_kernel_spmd(nc, in_maps, core_ids=list(range(N_CORES)),
                               trace=trace, tmpdir=tmpdir)
    return assemble(res.results), res


def kernel(**inputs):
    out, _ = _run(inputs)
    return out


# revision 39
# speedup vs baseline: 1.1347x; 1.1347x over previous
"""Multi-head attention (B=2, N=2048, C=1024, H=16, D=64) on 8 TRN2 NeuronCores.

Sharding: core c = (batch b = c//4) x (head-group g = c%4 -> heads 4g..4g+3).
Data parallel on B, tensor parallel on heads; fp16 ReduceScatter of the
out-projection partials within each 4-core batch group.

Everything on device stays transposed ([channel, position]); the host
pre-transposes inputs and post-transposes the output.
"""

import numpy as np
import ml_dtypes

import concourse.bacc as bacc
import concourse.tile as tile
import concourse.mybir as mybir
from concourse.bass_utils import run_bass_kernel_spmd

B, N, C, H = 2, 2048, 1024, 16
D = C // H          # 64
HL = H // 4         # 4 heads per core
CL = HL * D         # 256 local channels
N_CORES = 8
GROUPS = [[0, 1, 2, 3], [4, 5, 6, 7]]

F32 = mybir.dt.float32
BF16 = mybir.dt.float16
BF = np.float16

KC = C // 128       # 8  K-chunks of the input channel dim
NI = N // 512       # 4  512-wide i-chunks
NJ = N // 128       # 16 128-row j-chunks
IH = N // 1024      # 2  i-halves (attention granularity / RS chunks)


def build_kernel(n_cores=N_CORES, groups=GROUPS):
    group_size = len(groups[0])
    rs_out_rows = C // group_size

    nc = bacc.Bacc("TRN2", target_bir_lowering=False, debug=False,
                   num_devices=n_cores)

    xT = nc.declare_dram_parameter("xT", [C, N], BF16, isOutput=False)
    cos2 = nc.declare_dram_parameter("cos2", [128, N], BF16, isOutput=False)
    sin2s = nc.declare_dram_parameter("sin2s", [128, N], BF16, isOutput=False)
    wqkT = nc.declare_dram_parameter("wqkT", [C, 2 * CL], BF16, isOutput=False)
    bqk = nc.declare_dram_parameter("bqk", [2 * CL, 1], F32, isOutput=False)
    wvT = nc.declare_dram_parameter("wvT", [C, CL], BF16, isOutput=False)
    wprojT = nc.declare_dram_parameter("wprojT", [CL, C], BF16, isOutput=False)
    beff = nc.declare_dram_parameter("beff", [rs_out_rows, 1], F32, isOutput=False)
    out = nc.declare_dram_parameter("out", [rs_out_rows, N], F32, isOutput=True)

    with tile.TileContext(nc) as tc:
        with tc.tile_pool(name="dram", bufs=1, space="DRAM") as dram, \
             tc.tile_pool(name="sbuf", bufs=1) as sb, \
             tc.tile_pool(name="psum", bufs=1, space="PSUM") as ps:

            # ---- load inputs (wqk/xb interleaved so the qk matmuls can start
            # before the full x transfer lands) ----
            # both HWDGE queues (sync + scalar) share the bulk input load
            xb, wqk_sb = [], []
            for kc in range(KC):
                t = sb.tile([128, 2 * CL], BF16, name=f"wqk{kc}", tag=f"wqk{kc}")
                eng = nc.scalar if kc % 2 == 0 else nc.sync
                eng.dma_start(t[:], wqkT.ap()[128 * kc:128 * (kc + 1), :])
                wqk_sb.append(t)
                t = sb.tile([128, N], BF16, name=f"xb{kc}", tag=f"xb{kc}")
                eng = nc.sync if kc % 2 == 0 else nc.scalar
                eng.dma_start(t[:], xT.ap()[128 * kc:128 * (kc + 1), :])
                xb.append(t)
            wv_sb = []
            for kc in range(KC):
                t = sb.tile([128, CL], BF16, name=f"wv{kc}", tag=f"wv{kc}")
                nc.sync.dma_start(t[:], wvT.ap()[128 * kc:128 * (kc + 1), :])
                wv_sb.append(t)
            cos_sb = sb.tile([128, N], BF16, name="cos_sb", tag="cos_sb")
            nc.sync.dma_start(cos_sb[:], cos2.ap())
            sin_sb = sb.tile([128, N], BF16, name="sin_sb", tag="sin_sb")
            nc.scalar.dma_start(sin_sb[:], sin2s.ap())
            bqk_sb = []
            for m in range(4):
                t = sb.tile([128, 1], F32, name=f"bqk{m}", tag=f"bqk{m}")
                nc.sync.dma_start(t[:], bqk.ap()[128 * m:128 * (m + 1), :])
                bqk_sb.append(t)
            wproj_sb = []
            for p in range(2):
                t = sb.tile([128, C], BF16, name=f"wproj{p}", tag=f"wproj{p}")
                nc.sync.dma_start(t[:], wprojT.ap()[128 * p:128 * (p + 1), :])
                wproj_sb.append(t)
            beff_sb = []
            for m in range(rs_out_rows // 128):
                t = sb.tile([128, 1], F32, name=f"beff{m}", tag=f"beff{m}")
                nc.sync.dma_start(t[:], beff.ap()[128 * m:128 * (m + 1), :])
                beff_sb.append(t)

            # ---- qk projection + RoPE ----
            # chunk m rows: m=0:[q_h0,q_h1] m=1:[q_h2,q_h3] m=2:[k_h0,k_h1] m=3:[k_h2,k_h3]
            # so q and k of head h sit at the same partition offset 64*(h%2).
            # k of each head lands in its own zero-padded [128, N] tile so the
            # scores matmul can contract over K=128 (16-bit matmuls run at
            # half rate for K=64 -- zero rows buy back the full rate).
            q_r = []      # 2 tiles: [q_h0,q_h1], [q_h2,q_h3]
            k_t = []      # 4 tiles: k_h at rows 64*(h%2), zeros elsewhere
            for h in range(4):
                kt = sb.tile([128, N], BF16, name=f"ktile{h}", tag=f"ktile{h}")
                z = slice(0, 64) if h % 2 == 1 else slice(64, 128)
                nc.vector.memset(kt[z, :], 0.0)
                k_t.append(kt)
            swap_mask = [i ^ 1 for i in range(32)]
            # kc-outer accumulation so the first matmul only needs the first
            # x/w chunk off DMA; 2 PSUM tiles hold the 4 m-accumulators
            qks_t = [sb.tile([128, N], BF16, name=f"qks{m}", tag=f"qks{m}")
                     for m in range(4)]
            for n in range(NI):
                accs = [ps.tile([128, 1024], F32, name=f"qacc{n}_{a}",
                                tag="sc", bufs=2) for a in range(2)]
                for kc in range(KC):
                    for m in range(4):
                        nc.tensor.matmul(
                            accs[m // 2][:, 512 * (m % 2):512 * (m % 2 + 1)],
                            wqk_sb[kc][:, 128 * m:128 * (m + 1)],
                            xb[kc][:, 512 * n:512 * (n + 1)],
                            start=(kc == 0), stop=(kc == KC - 1))
                for m in range(4):
                    nc.scalar.activation(
                        qks_t[m][:, 512 * n:512 * (n + 1)],
                        accs[m // 2][:, 512 * (m % 2):512 * (m % 2 + 1)],
                        mybir.ActivationFunctionType.Identity,
                        bias=bqk_sb[m][:])
            for m in range(4):
                qks = qks_t[m]
                # RoPE: qk' = qks*cos2 + shift(qks)*sin2s
                # (pair-swap of adjacent partitions via DVE stream shuffle)
                shf = sb.tile([128, N], BF16, name=f"shf{m}", tag="shf", bufs=2)
                nc.vector.stream_shuffle(shf[:], qks[:], swap_mask)
                t2 = sb.tile([128, N], BF16, name=f"ropetmp{m}", tag="ropetmp", bufs=2)
                nc.vector.tensor_mul(t2[:], shf[:], sin_sb[:])
                if m < 2:
                    qkr = sb.tile([128, N], BF16, name=f"qkr{m}", tag=f"qkr{m}")
                    nc.vector.tensor_mul(qkr[:], qks[:], cos_sb[:])
                    nc.vector.tensor_add(qkr[:], qkr[:], t2[:])
                    q_r.append(qkr)
                else:
                    t1 = sb.tile([128, N], BF16, name=f"ropetc{m}", tag="ropetc",
                                 bufs=2)
                    nc.vector.tensor_mul(t1[:], qks[:], cos_sb[:])
                    h0, h1 = 2 * (m - 2), 2 * (m - 2) + 1
                    nc.vector.tensor_add(k_t[h0][0:64, :], t1[0:64, :],
                                         t2[0:64, :])
                    nc.vector.tensor_add(k_t[h1][64:128, :], t1[64:128, :],
                                         t2[64:128, :])

            # ---- v projection (natural [j, ch] layout, ones col appended per head) ----
            vaug = []
            for jc in range(NJ):
                pv = ps.tile([128, CL], F32, name=f"pv{jc}", tag="sc", bufs=2)
                for kc in range(KC):
                    nc.tensor.matmul(
                        pv[:],
                        xb[kc][:, 128 * jc:128 * (jc + 1)],
                        wv_sb[kc][:],
                        start=(kc == 0), stop=(kc == KC - 1))
                va = sb.tile([128, HL * (D + 1)], BF16, name=f"vaug{jc}",
                             tag=f"vaug{jc}")
                nc.vector.memset(va[:, D::D + 1], 1.0)
                nc.scalar.activation(
                    va.rearrange("p (h e) -> p h e", e=D + 1)[:, :, 0:D],
                    pv.rearrange("p (h e) -> p h e", e=D)[:, :, :],
                    mybir.ActivationFunctionType.Copy)
                vaug.append(va)

            # per-partition bias AP used to shift scores before fp16 exp
            eshift = sb.tile([128, 1], F32, name="eshift", tag="eshift")
            nc.vector.memset(eshift[:], -16.0)
            # K=1 ones row used to broadcast denominators across partitions
            ones64 = sb.tile([1, 64], BF16, name="ones64", tag="ones64")
            nc.vector.memset(ones64[:], 1.0)

            # ---- attention + projection + RS, per i-half ----
            rs_outs = []
            for ih in range(IH):
                i0 = 1024 * ih
                o_pair = [sb.tile([128, 1024], BF16, name=f"opair{ih}_{p}",
                                  tag=f"opair{p}", bufs=2) for p in range(2)]
                for hl in range(4):
                    qT = q_r[hl // 2]
                    kT = k_t[hl]
                    oacc = ps.tile([65, 1024], F32, name=f"oacc{ih}_{hl}",
                                   tag="oacc", bufs=2)
                    exs = []

                    def emit_o(jc):
                        for q in range(2):
                            nc.tensor.matmul(
                                oacc[:, 512 * q:512 * (q + 1)],
                                vaug[jc][:, (D + 1) * hl:(D + 1) * (hl + 1)],
                                exs[jc][:, 512 * q:512 * (q + 1)],
                                start=(jc == 0), stop=(jc == NJ - 1))

                    for jc in range(NJ):
                        sc = ps.tile([128, 1024], F32, name=f"sc{ih}_{hl}_{jc}",
                                     tag="sc", bufs=2)
                        for q in range(2):
                            nc.tensor.matmul(
                                sc[:, 512 * q:512 * (q + 1)],
                                kT[:, 128 * jc:128 * (jc + 1)],
                                qT[:, i0 + 512 * q:i0 + 512 * (q + 1)],
                                start=True, stop=True)
                        ex = sb.tile([128, 1024], BF16, name=f"ex{ih}_{hl}_{jc}",
                                     tag="ex", bufs=3)
                        # bias shifts all scores so fp16 exp can't overflow
                        # (softmax is shift-invariant, cancels in num/den)
                        nc.scalar.activation(ex[:], sc[:],
                                             mybir.ActivationFunctionType.Exp,
                                             scale=float(1.0 / np.sqrt(D)),
                                             bias=eshift[:])
                        exs.append(ex)
                        # software pipeline: o-matmuls lag one j-chunk so the
                        # PE never sits waiting on the exp of the current one
                        if jc >= 1:
                            emit_o(jc - 1)
                    emit_o(NJ - 1)
                    # normalize: o[:, i] / den[i]
                    # broadcast den across partitions with a K=1 matmul
                    # (no DMA: DMA triggers on any engine queue can block it
                    # while collective SDMA traffic is in flight), then
                    # reciprocal+mul on 64 partitions
                    den = sb.tile([1, 1024], BF16, name=f"den{ih}_{hl}",
                                  tag="den", bufs=2)
                    nc.scalar.activation(den[:], oacc[64:65, :],
                                         mybir.ActivationFunctionType.Copy)
                    rb = ps.tile([64, 1024], F32, name=f"rb{ih}_{hl}",
                                 tag="oacc", bufs=2)
                    for q in range(2):
                        nc.tensor.matmul(rb[:, 512 * q:512 * (q + 1)],
                                         ones64[:],
                                         den[:, 512 * q:512 * (q + 1)],
                                         start=True, stop=True)
                    rr = sb.tile([64, 1024], F32, name=f"rr{ih}_{hl}", tag="rr",
                                 bufs=2)
                    nc.vector.reciprocal_approx_fast(rr[:], rb[:])
                    nc.vector.tensor_mul(
                        o_pair[hl // 2][64 * (hl % 2):64 * (hl % 2) + 64, :],
                        oacc[0:64, :], rr[:])

                # out-projection partial for this i-half
                rs_in = dram.tile([C, 1024], BF16, name=f"rsin{ih}", tag=f"rsin{ih}")
                for n2 in range(2):
                    isl = slice(512 * n2, 512 * (n2 + 1))
                    for mc in range(KC):
                        pp = ps.tile([128, 512], F32, name=f"pp{ih}_{n2}_{mc}",
                                     tag="sc" if mc % 2 == 0 else "oacc",
                                     bufs=2)
                        for p in range(2):
                            nc.tensor.matmul(
                                pp[:],
                                wproj_sb[p][:, 128 * mc:128 * (mc + 1)],
                                o_pair[p][:, isl],
                                start=(p == 0), stop=(p == 1))
                        po = sb.tile([128, 512], BF16, name=f"po{ih}_{n2}_{mc}",
                                     tag="po", bufs=4)
                        # alternate evict engine so PSUM slots recycle 2x faster
                        if mc % 2 == 0:
                            nc.vector.tensor_copy(po[:], pp[:])
                        else:
                            nc.scalar.activation(
                                po[:], pp[:],
                                mybir.ActivationFunctionType.Copy)
                        nc.sync.dma_start(
                            rs_in[128 * mc:128 * (mc + 1), isl], po[:])
                rs_out = dram.tile([rs_out_rows, 1024], BF16, name=f"rsout{ih}",
                                   tag=f"rsout{ih}")
                nc.gpsimd.collective_compute(
                    "ReduceScatter", mybir.AluOpType.add,
                    replica_groups=groups,
                    ins=[rs_in[:]], outs=[rs_out[:]])
                rs_outs.append(rs_out)

            # ---- RS result + bias -> output ----
            # gpsimd DMA queue: keeps RS-dependent reads out of the sync
            # queue so they can't head-of-line-block compute-feeding DMAs
            for ih in range(IH):
                for m in range(rs_out_rows // 128):
                    rbk = sb.tile([128, 1024], BF16, name=f"rbk{ih}_{m}",
                                  tag="rbk", bufs=2)
                    nc.gpsimd.dma_start(
                        rbk[:], rs_outs[ih][128 * m:128 * (m + 1), :])
                    fo = sb.tile([128, 1024], F32, name=f"fo{ih}_{m}", tag="fo",
                                 bufs=2)
                    nc.vector.tensor_scalar_add(fo[:], rbk[:], beff_sb[m][:])
                    nc.gpsimd.dma_start(
                        out.ap()[128 * m:128 * (m + 1),
                                 1024 * ih:1024 * (ih + 1)], fo[:])

    nc.compile()
    return nc


def shard_inputs(x, rope, w_qkv, b_qkv, w_proj, b_proj,
                 n_cores=N_CORES, group_size=4):
    """Per-core input maps. Host-side transposes/casts are part of sharding."""
    rs_out_rows = C // group_size
    # fold the v-bias through the projection into an effective output bias
    b_v = b_qkv[2 * C:3 * C]
    b_eff = (b_proj + b_v @ w_proj.T).astype(np.float32)   # [C]

    in_maps = []
    for c in range(n_cores):
        b = (c // group_size) % B
        g = c % group_size
        heads = range(HL * g, HL * g + HL)

        xTb = np.ascontiguousarray(x[b].T).astype(BF)            # [C, N]

        cosT = rope[b].T[:D, :]                                   # [64, N]
        sinT = rope[b].T[D:, :]
        cos2 = np.vstack([cosT, cosT]).astype(BF)                 # [128, N]
        sgn = np.where(np.arange(128) % 2 == 0, -1.0, 1.0)[:, None]
        sin2s = (np.vstack([sinT, sinT]) * sgn).astype(BF)        # [128, N]

        # qk weight rows ordered [q_h0..q_h3, k_h0..k_h3]
        qk_rows = []
        bqk_rows = []
        for h in heads:
            qk_rows.append(w_qkv[D * h:D * (h + 1), :])           # q rows
            bqk_rows.append(b_qkv[D * h:D * (h + 1)])
        for h in heads:
            qk_rows.append(w_qkv[C + D * h:C + D * (h + 1), :])   # k rows
            bqk_rows.append(b_qkv[C + D * h:C + D * (h + 1)])
        wqk = np.vstack(qk_rows)                                  # [512, C]
        wqkT = np.ascontiguousarray(wqk.T).astype(BF)             # [C, 512]
        bqk_v = np.concatenate(bqk_rows).astype(np.float32)[:, None]

        h0 = HL * g
        wv = w_qkv[2 * C + D * h0:2 * C + D * h0 + CL, :]          # [256, C]
        wvT = np.ascontiguousarray(wv.T).astype(BF)                # [C, 256]

        wp = w_proj[:, D * h0:D * h0 + CL]                         # [C, 256]
        wprojT = np.ascontiguousarray(wp.T).astype(BF)             # [256, C]

        r = c % group_size
        beff_shard = b_eff[rs_out_rows * r:rs_out_rows * (r + 1)].astype(
            np.float32)[:, None]

        in_maps.append({
            "xT": xTb, "cos2": cos2, "sin2s": sin2s,
            "wqkT": wqkT, "bqk": bqk_v, "wvT": wvT,
            "wprojT": wprojT, "beff": beff_shard,
        })
    return in_maps


def assemble(results, n_cores=N_CORES, group_size=4):
    rs_out_rows = C // group_size
    out = np.empty((B, N, C), dtype=np.float32)
    for c in range(n_cores):
        b = (c // group_size) % B
        r = c % group_size
        outT_shard = results[c]["out"]                 # [rs_out_rows, N] f32
        out[b, :, rs_out_rows * r:rs_out_rows * (r + 1)] = outT_shard.T
    return out


_NC_CACHE = {}


def _get_nc():
    if "nc" not in _NC_CACHE:
        _NC_CACHE["nc"] = build_kernel()
    return _NC_CACHE["nc"]


def _run(inputs, trace=False, tmpdir=None):
    nc = _get_nc()
    in_maps = shard_inputs(**inputs)
    res = run_bass_kernel_spmd(nc, in_maps, core_ids=list(range(N_CORES)),
                               trace=trace, tmpdir=tmpdir)
    return assemble(res.results), res


def kernel(**inputs):
    out, _ = _run(inputs)
    return out
